# revision 9
# baseline (speedup 1.0000x reference)
"""Trainium2 Bass kernel for nn_CLIP topk_masking.

Computes, for full inputs (self-contained; shapes hardcoded):
    probability = image_features @ ima_proto.T          # [B, NP]
    thr_r       = k-th largest of probability row r
    sel[r, j]   = probability[r, j] >= thr_r            # top-k prototype mask
    text_n      = exp(logit_scale) * text_raw / ||text_raw||_row
    logits[r,c] = (image_features @ text_n.T)[r,c] * sel[r, c // 10]

Sharding: data-parallel over the batch axis across 8 NeuronCores
(rows 512/core); prototypes and text features replicated.

v2 design notes (vs the earlier baseline):
  - Text SBUF tiles use 80 partitions (10000 = 125*80) so every DMA fans
    out across all 16 SDMA engines (125-partition tiles only got 5).
  - Text is cast f32->bf16 during the SWDGE load; the whole text pipeline
    (norms, scale, transpose, logit matmul) runs in bf16.
  - Per-class normalization (and exp(logit_scale)) is folded into the PE
    transpose: the transpose's moving operand is diag(rcp) instead of I.
  - Row norms on ACT (Square + accum_out, one activation table), rsqrt on
    ACT, diag build on GPSIMD, PSUM drains + mask-apply on DVE.
  - Output stores go on the HWDGE (sync) queue, text loads on SWDGE.
"""

import os
from contextlib import ExitStack

import numpy as np

import concourse.bass as bass
import concourse.tile as tile
from concourse import bacc, mybir
from concourse.bass_utils import run_bass_kernel_spmd

# Problem shapes (hardcoded per contract).
B, D, NP, NC, CPT = 4096, 512, 1000, 10000, 10
NCORES = 8
RLOC = B // NCORES          # 512 rows per core
RT = RLOC // 128            # 4 row tiles per core
KD = D // 128               # 4 contraction chunks
CT = 80                     # text classes per tile (80-partition DMA tiles)
TPC = 5                     # text tiles per chunk
CHW = CT * TPC              # 400 classes per logit-matmul chunk
NCH = NC // CHW             # 25 chunks
GRP = 5                     # chunks per output stage group (2000 cols/store)
CTP = 100                   # proto classes per tile (10-engine DMA tiles)
UNIT = 800                  # text classes per load unit (2 chunks)
NUNIT = 13                  # 12 full units + one 400-class tail
NEG = -1.0e30

F32 = mybir.dt.float32
BF16 = mybir.dt.bfloat16

LAST_RESULTS = None


def _emit(ctx: ExitStack, tc, img, proto, text, out, k: int, inv_s2: float):
    nc = tc.nc
    AF = mybir.ActivationFunctionType
    OP = mybir.AluOpType

    const = ctx.enter_context(tc.tile_pool(name="const", bufs=1))
    persist = ctx.enter_context(tc.tile_pool(name="persist", bufs=1))

    # Identity matrices for PE transposes (f32 for img/proto, bf16 base for
    # the per-tile diag(rcp) used by the text transpose).
    ones = const.tile([128, 128], F32)
    nc.vector.memset(ones[:], 1.0)
    ident = const.tile([128, 128], F32)
    nc.gpsimd.affine_select(
        ident[:], ones[:], pattern=[[1, 128]], compare_op=OP.is_equal,
        fill=0.0, base=0, channel_multiplier=-1,
    )
    identb = const.tile([CT, CT], BF16)
    nc.vector.tensor_copy(identb[:], ident[:CT, :CT])

    # imgT[p, kc, r] = img[r, kc*128 + p]; f32 for the prob matmul, bf16
    # copy for the logit matmul. sel[rt] = top-k prototype mask per row.
    imgT = persist.tile([128, KD, RLOC], F32)
    imgTb = persist.tile([128, KD, RLOC], BF16)
    sels = []

    # Text loads: SWDGE (gpsimd) with f32->bf16 cast, [80, n, 512] tiles so
    # all 16 SDMA engines participate. 5 buffers so a load never has to
    # wait on its buffer's previous consumers (no Pool-queue stalls).
    pb_traw = ctx.enter_context(tc.tile_pool(name="pb_traw", bufs=5))
    traw_tiles = {}

    def load_unit(u: int):
        rows = UNIT if u < NUNIT - 1 else NC - UNIT * (NUNIT - 1)
        t_ = pb_traw.tile([CT, rows // CT, D], BF16, name=f"traw{u}", tag="traw")
        nc.gpsimd.dma_start(
            t_[:], text[u * UNIT:u * UNIT + rows].rearrange(
                "(t p) d -> p t d", p=CT))
        traw_tiles[u] = t_

    for u in range(4):
        load_unit(u)

    # Long-lived text-pipeline pools (PSUM: ttT 2 banks x 2 bufs = 4).
    pb_nrm = ctx.enter_context(tc.tile_pool(name="pb_nrm", bufs=3))
    pb_diag = ctx.enter_context(tc.tile_pool(name="pb_diag", bufs=2 * TPC))
    pb_ttT_ps = ctx.enter_context(
        tc.tile_pool(name="pb_ttT_ps", bufs=1, space="PSUM"))
    pb_ttT = ctx.enter_context(tc.tile_pool(name="pb_ttT", bufs=3))
    dump = const.tile([CT, D], BF16)   # activation main-out scratch

    # ---------- Phase A: img/proto transpose, probability matmul, top-k ----------
    with (
        tc.tile_pool(name="pa_sb", bufs=1) as pa_sb,
        tc.tile_pool(name="pa_ps", bufs=2, space="PSUM") as pa_ps,
        tc.tile_pool(name="pa_prob_ps", bufs=2, space="PSUM") as pa_prob_ps,
        tc.tile_pool(name="pa_work", bufs=2) as pa_work,
    ):
        # img/proto on the HWDGE (sync) queue; text owns SWDGE.
        img_sb = pa_sb.tile([128, RT, D], F32)
        nc.sync.dma_start(img_sb[:], img.rearrange("(t p) d -> p t d", p=128))
        for rt in range(RT):
            pi = pa_ps.tile([128, KD, 128], F32, tag="pt")
            for kc in range(KD):
                nc.tensor.transpose(
                    pi[:, kc], img_sb[:, rt, kc * 128:(kc + 1) * 128], ident[:])
            nc.vector.tensor_copy(imgT[:, :, rt * 128:(rt + 1) * 128], pi[:])
        nc.vector.tensor_copy(imgTb[:], imgT[:])

        proto_sb = pa_sb.tile([CTP, NP // CTP, D], F32)
        nc.sync.dma_start(proto_sb[:], proto.rearrange("(t p) d -> p t d", p=CTP))
        protoT = pa_sb.tile([128, KD, NP], F32)
        for t in range(NP // CTP):
            pp = pa_ps.tile([128, KD, 128], F32, tag="pt")
            for kc in range(KD):
                nc.tensor.transpose(
                    pp[:, kc, :CTP], proto_sb[:, t, kc * 128:(kc + 1) * 128],
                    ident[:CTP, :CTP])
            nc.vector.tensor_copy(
                protoT[:, :, t * CTP:(t + 1) * CTP], pp[:, :, :CTP])

        for rt in range(RT):
            prob = pa_work.tile([128, NP], F32, tag="prob")
            for h in range(2):
                ppr = pa_prob_ps.tile([128, NP // 2], F32, tag="ppr")
                for kc in range(KD):
                    # fp32 (not bf16): ranking precision decides the mask.
                    nc.tensor.matmul(
                        ppr[:],
                        imgT[:, kc, rt * 128:(rt + 1) * 128],
                        protoT[:, kc, h * (NP // 2):(h + 1) * (NP // 2)],
                        start=(kc == 0), stop=(kc == KD - 1),
                    )
                nc.vector.tensor_copy(prob[:, h * (NP // 2):(h + 1) * (NP // 2)], ppr[:])
            m8a = pa_work.tile([128, 8], F32, tag="m8a")
            nc.vector.max(m8a[:], prob[:])
            if k <= 8:
                thr = m8a[:, k - 1:k]
            else:
                repl = pa_work.tile([128, NP], F32, tag="repl")
                nc.vector.match_replace(repl[:], m8a[:], prob[:], NEG)
                m8b = pa_work.tile([128, 8], F32, tag="m8b")
                nc.vector.max(m8b[:], repl[:])
                thr = m8b[:, k - 9:k - 8]
            sel = persist.tile([128, NP], F32, tag=f"sel{rt}")
            # sel build on GPSIMD to keep DVE free for PSUM drains.
            nc.gpsimd.tensor_scalar(sel[:], prob[:], thr, None, op0=OP.is_ge)
            sels.append(sel)

    # ---------- Phase B: text norms, scaled transpose, logit matmul, mask ----------
    def emit_norms(c: int):
        u, half = divmod(c, 2)
        traw = traw_tiles[u]
        toff = half * TPC
        nrm = pb_nrm.tile([CT, TPC], F32, tag="nrm", name=f"nrm{c}")
        for t in range(TPC):
            nc.scalar.activation(
                dump[:], traw[:, toff + t], AF.Square,
                accum_out=nrm[:, t:t + 1])
        nrs = pb_nrm.tile([CT, TPC], F32, tag="nrs", name=f"nrs{c}")
        # sqrt(||t||^2 * exp(-2*logit_scale)) = ||t|| / exp(logit_scale)
        nc.scalar.activation(nrs[:], nrm[:], AF.Sqrt, scale=inv_s2)
        rcp = pb_nrm.tile([CT, TPC], F32, tag="rcp", name=f"rcp{c}")
        nc.vector.reciprocal(rcp[:], nrs[:])
        return rcp

    rcps = {0: emit_norms(0), 1: emit_norms(1)}

    with (
        tc.tile_pool(name="pb_psM", bufs=RT, space="PSUM") as pb_psM,
        tc.tile_pool(name="pb_stage", bufs=2) as pb_stage,
    ):
        stages = [None] * RT
        ttTs = {}

        def emit_logit(c: int):
            g, pos = divmod(c, GRP)
            ttT = ttTs.pop(c)
            for rt in range(RT):
                pm = pb_psM.tile([128, CHW], F32, tag="pm")
                for kc in range(KD):
                    nc.tensor.matmul(
                        pm[:],
                        imgTb[:, kc, rt * 128:(rt + 1) * 128],
                        ttT[:, kc],
                        start=(kc == 0), stop=(kc == KD - 1),
                    )
                if pos == 0:
                    stages[rt] = pb_stage.tile(
                        [128, GRP * CHW], F32, tag=f"stg{rt}", name=f"stg{rt}g{g}")
                selb = sels[rt][:, c * (CHW // CPT):(c + 1) * (CHW // CPT)]
                selb = selb.broadcast_to([128, CHW // CPT, CPT])
                dst = stages[rt][:, pos * CHW:(pos + 1) * CHW]
                nc.vector.tensor_tensor(
                    dst.rearrange("p (a b) -> p a b", b=CPT),
                    pm[:].rearrange("p (a b) -> p a b", b=CPT),
                    selb, op=OP.mult)
                if pos == GRP - 1:
                    nc.sync.dma_start(
                        out[rt * 128:(rt + 1) * 128,
                            g * GRP * CHW:(g + 1) * GRP * CHW],
                        stages[rt][:])

        for c in range(NCH):
            if c + 2 < NCH:
                rcps[c + 2] = emit_norms(c + 2)
            if c % 2 == 0 and c // 2 + 4 < NUNIT:
                load_unit(c // 2 + 4)

            u, half = divmod(c, 2)
            traw = traw_tiles[u]
            toff = half * TPC
            rcp = rcps.pop(c)

            # Per-tile diag(rcp) on GPSIMD, then a REGULAR matmul computes the
            # scaled transpose: ttT[:, kc, tile] = traw_tile[:, kc].T @ diag.
            # (PE transpose-mode ignores the values of its moving operand, so
            # it cannot apply the scaling; a plain matmul at the same cost can.)
            # Padded to 512/kc-row (one full PSUM bank) so no matmul output
            # crosses a 2KB bank boundary (crossing corrupts the write).
            ttT_ps = pb_ttT_ps.tile([128, KD, 512], F32, tag="ttps")
            for t in range(TPC):
                diag = pb_diag.tile([CT, CT], BF16, tag="diag")
                nc.gpsimd.tensor_scalar(
                    diag[:], identb[:], rcp[:, t:t + 1], None, op0=OP.mult)
                for kc in range(KD):
                    nc.tensor.matmul(
                        ttT_ps[:, kc, t * CT:(t + 1) * CT],
                        traw[:, toff + t, kc * 128:(kc + 1) * 128],
                        diag[:], start=True, stop=True)
            ttT = pb_ttT.tile([128, KD, CHW], BF16, tag="ttT")
            # f32 PSUM -> bf16 SBUF cast-copy; alternate ACT/DVE for balance.
            if c % 2 == 0:
                nc.scalar.copy(ttT[:], ttT_ps[:, :, :CHW])
            else:
                nc.vector.tensor_copy(ttT[:], ttT_ps[:, :, :CHW])
            ttTs[c] = ttT

            # Lag the logit matmuls one chunk behind the transposes so the
            # PE never waits on the DVE drain of the chunk it just produced.
            if c > 0:
                emit_logit(c - 1)
        emit_logit(NCH - 1)


def _build(k: int, inv_s2: float):
    nc = bacc.Bacc("TRN2", target_bir_lowering=False, debug=False)
    img = nc.dram_tensor("img", [RLOC, D], F32, kind="ExternalInput").ap()
    proto = nc.dram_tensor("proto", [NP, D], F32, kind="ExternalInput").ap()
    text = nc.dram_tensor("text", [NC, D], F32, kind="ExternalInput").ap()
    out = nc.dram_tensor("out", [RLOC, NC], F32, kind="ExternalOutput").ap()
    with tile.TileContext(nc) as tc:
        with ExitStack() as ctx:
            _emit(ctx, tc, img, proto, text, out, k, inv_s2)
    nc.compile()
    return nc


def kernel(image_features, ima_proto, text_features_raw, logit_scale, num_test):
    global LAST_RESULTS
    img = np.ascontiguousarray(np.asarray(image_features, dtype=np.float32))
    proto = np.ascontiguousarray(np.asarray(ima_proto, dtype=np.float32))
    text = np.ascontiguousarray(np.asarray(text_features_raw, dtype=np.float32))
    assert img.shape == (B, D) and proto.shape == (NP, D) and text.shape == (NC, D)
    s = float(np.asarray(logit_scale))
    k = min(int(np.asarray(num_test)), NP)
    assert 1 <= k <= 16, f"kernel supports k in [1, 16], got {k}"
    inv_s2 = float(np.exp(-2.0 * s))

    nc = _build(k, inv_s2)
    in_maps = [
        {"img": img[i * RLOC:(i + 1) * RLOC], "proto": proto, "text": text}
        for i in range(NCORES)
    ]
    trace = bool(int(os.environ.get("BASS_KERNEL_TRACE", "0")))
    res = run_bass_kernel_spmd(nc, in_maps, list(range(NCORES)), trace=trace)
    LAST_RESULTS = res
    return np.concatenate([r["out"] for r in res.results], axis=0)


# revision 10
# speedup vs baseline: 1.4365x; 1.4365x over previous
"""Trainium2 Bass kernel for nn_CLIP topk_masking.

Computes, for full inputs (self-contained; shapes hardcoded):
    probability = image_features @ ima_proto.T          # [B, NP]
    thr_r       = k-th largest of probability row r
    sel[r, j]   = probability[r, j] >= thr_r            # top-k prototype mask
    text_n      = exp(logit_scale) * text_raw / ||text_raw||_row
    logits[r,c] = (image_features @ text_n.T)[r,c] * sel[r, c // 10]

Sharding: data-parallel over the batch axis across 8 NeuronCores
(rows 512/core); prototypes and text features replicated.

v2 design notes (vs the earlier baseline):
  - Text SBUF tiles use 80 partitions (10000 = 125*80) so every DMA fans
    out across all 16 SDMA engines (125-partition tiles only got 5).
  - Text is cast f32->bf16 during the SWDGE load; the whole text pipeline
    (norms, scale, transpose, logit matmul) runs in bf16.
  - Per-class normalization (and exp(logit_scale)) is folded into the PE
    transpose: the transpose's moving operand is diag(rcp) instead of I.
  - Row norms on ACT (Square + accum_out, one activation table), rsqrt on
    ACT, diag build on GPSIMD, PSUM drains + mask-apply on DVE.
  - Output stores go on the HWDGE (sync) queue, text loads on SWDGE.
"""

import os
from contextlib import ExitStack

import numpy as np

import concourse.bass as bass
import concourse.tile as tile
from concourse import bacc, mybir
from concourse.bass_utils import run_bass_kernel_spmd

# Problem shapes (hardcoded per contract).
B, D, NP, NC, CPT = 4096, 512, 1000, 10000, 10
NCORES = 8
RLOC = B // NCORES          # 512 rows per core
RT = RLOC // 128            # 4 row tiles per core
KD = D // 128               # 4 contraction chunks
CT = 80                     # text classes per tile (80-partition DMA tiles)
TPC = 5                     # text tiles per chunk
CHW = CT * TPC              # 400 classes per logit-matmul chunk
NCH = NC // CHW             # 25 chunks
GRP = 5                     # chunks per output stage group (2000 cols/store)
CTP = 100                   # proto classes per tile (10-engine DMA tiles)
UNIT = 800                  # text classes per load unit (2 chunks)
NUNIT = 13                  # 12 full units + one 400-class tail
NEG = -1.0e30

F32 = mybir.dt.float32
BF16 = mybir.dt.bfloat16

LAST_RESULTS = None


def _emit(ctx: ExitStack, tc, img, proto, text, out, k: int, inv_s2: float):
    nc = tc.nc
    AF = mybir.ActivationFunctionType
    OP = mybir.AluOpType

    const = ctx.enter_context(tc.tile_pool(name="const", bufs=1))
    persist = ctx.enter_context(tc.tile_pool(name="persist", bufs=1))

    # Identity matrices for PE transposes (f32 for img/proto, bf16 base for
    # the per-tile diag(rcp) used by the text transpose).
    ones = const.tile([128, 128], F32)
    nc.vector.memset(ones[:], 1.0)
    ident = const.tile([128, 128], F32)
    nc.gpsimd.affine_select(
        ident[:], ones[:], pattern=[[1, 128]], compare_op=OP.is_equal,
        fill=0.0, base=0, channel_multiplier=-1,
    )
    identb = const.tile([CT, CT], BF16)
    nc.vector.tensor_copy(identb[:], ident[:CT, :CT])

    # imgT[p, kc, r] = img[r, kc*128 + p]; f32 for the prob matmul, bf16
    # copy for the logit matmul. sel[rt] = top-k prototype mask per row.
    imgT = persist.tile([128, KD, RLOC], F32)
    imgTb = persist.tile([128, KD, RLOC], BF16)
    sels = []

    # Text loads: SWDGE (gpsimd) with f32->bf16 cast, [80, n, 512] tiles so
    # all 16 SDMA engines participate. 5 buffers so a load never has to
    # wait on its buffer's previous consumers (no Pool-queue stalls).
    pb_traw = ctx.enter_context(tc.tile_pool(name="pb_traw", bufs=5))
    traw_tiles = {}

    def load_unit(u: int):
        rows = UNIT if u < NUNIT - 1 else NC - UNIT * (NUNIT - 1)
        t_ = pb_traw.tile([CT, rows // CT, D], BF16, name=f"traw{u}", tag="traw")
        nc.gpsimd.dma_start(
            t_[:], text[u * UNIT:u * UNIT + rows].rearrange(
                "(t p) d -> p t d", p=CT))
        traw_tiles[u] = t_

    for u in range(4):
        load_unit(u)

    # Long-lived text-pipeline pools (PSUM: ttT 2 banks x 2 bufs = 4).
    pb_nrm = ctx.enter_context(tc.tile_pool(name="pb_nrm", bufs=3))
    pb_diag = ctx.enter_context(tc.tile_pool(name="pb_diag", bufs=2 * TPC))
    pb_ttT_ps = ctx.enter_context(
        tc.tile_pool(name="pb_ttT_ps", bufs=1, space="PSUM"))
    pb_ttT = ctx.enter_context(tc.tile_pool(name="pb_ttT", bufs=3))
    dump = const.tile([CT, D], BF16)   # activation main-out scratch

    # ---------- Phase A: img/proto transpose, probability matmul, top-k ----------
    with (
        tc.tile_pool(name="pa_sb", bufs=1) as pa_sb,
        tc.tile_pool(name="pa_ps", bufs=2, space="PSUM") as pa_ps,
        tc.tile_pool(name="pa_prob_ps", bufs=2, space="PSUM") as pa_prob_ps,
        tc.tile_pool(name="pa_work", bufs=2) as pa_work,
    ):
        # img/proto on the HWDGE (sync) queue; text owns SWDGE.
        img_sb = pa_sb.tile([128, RT, D], F32)
        nc.sync.dma_start(img_sb[:], img.rearrange("(t p) d -> p t d", p=128))
        for rt in range(RT):
            pi = pa_ps.tile([128, KD, 128], F32, tag="pt")
            for kc in range(KD):
                nc.tensor.transpose(
                    pi[:, kc], img_sb[:, rt, kc * 128:(kc + 1) * 128], ident[:])
            nc.vector.tensor_copy(imgT[:, :, rt * 128:(rt + 1) * 128], pi[:])
        nc.vector.tensor_copy(imgTb[:], imgT[:])

        proto_sb = pa_sb.tile([CTP, NP // CTP, D], F32)
        nc.sync.dma_start(proto_sb[:], proto.rearrange("(t p) d -> p t d", p=CTP))
        protoT = pa_sb.tile([128, KD, NP], F32)
        for t in range(NP // CTP):
            pp = pa_ps.tile([128, KD, 128], F32, tag="pt")
            for kc in range(KD):
                nc.tensor.transpose(
                    pp[:, kc, :CTP], proto_sb[:, t, kc * 128:(kc + 1) * 128],
                    ident[:CTP, :CTP])
            nc.vector.tensor_copy(
                protoT[:, :, t * CTP:(t + 1) * CTP], pp[:, :, :CTP])

        for rt in range(RT):
            prob = pa_work.tile([128, NP], F32, tag="prob")
            for h in range(2):
                ppr = pa_prob_ps.tile([128, NP // 2], F32, tag="ppr")
                for kc in range(KD):
                    # fp32 (not bf16): ranking precision decides the mask.
                    nc.tensor.matmul(
                        ppr[:],
                        imgT[:, kc, rt * 128:(rt + 1) * 128],
                        protoT[:, kc, h * (NP // 2):(h + 1) * (NP // 2)],
                        start=(kc == 0), stop=(kc == KD - 1),
                    )
                nc.vector.tensor_copy(prob[:, h * (NP // 2):(h + 1) * (NP // 2)], ppr[:])
            m8a = pa_work.tile([128, 8], F32, tag="m8a")
            nc.vector.max(m8a[:], prob[:])
            if k <= 8:
                thr = m8a[:, k - 1:k]
            else:
                repl = pa_work.tile([128, NP], F32, tag="repl")
                nc.vector.match_replace(repl[:], m8a[:], prob[:], NEG)
                m8b = pa_work.tile([128, 8], F32, tag="m8b")
                nc.vector.max(m8b[:], repl[:])
                thr = m8b[:, k - 9:k - 8]
            sel = persist.tile([128, NP], F32, tag=f"sel{rt}")
            # sel build on GPSIMD to keep DVE free for PSUM drains.
            nc.gpsimd.tensor_scalar(sel[:], prob[:], thr, None, op0=OP.is_ge)
            sels.append(sel)

    # ---------- Phase B: text norms, scaled transpose, logit matmul, mask ----------
    def emit_norms(c: int):
        u, half = divmod(c, 2)
        traw = traw_tiles[u]
        toff = half * TPC
        nrm = pb_nrm.tile([CT, TPC], F32, tag="nrm", name=f"nrm{c}")
        for t in range(TPC):
            nc.scalar.activation(
                dump[:], traw[:, toff + t], AF.Square,
                accum_out=nrm[:, t:t + 1])
        nrs = pb_nrm.tile([CT, TPC], F32, tag="nrs", name=f"nrs{c}")
        # sqrt(||t||^2 * exp(-2*logit_scale)) = ||t|| / exp(logit_scale)
        nc.scalar.activation(nrs[:], nrm[:], AF.Sqrt, scale=inv_s2)
        rcp = pb_nrm.tile([CT, TPC], F32, tag="rcp", name=f"rcp{c}")
        nc.vector.reciprocal(rcp[:], nrs[:])
        return rcp

    rcps = {0: emit_norms(0), 1: emit_norms(1)}

    with (
        tc.tile_pool(name="pb_psM", bufs=RT, space="PSUM") as pb_psM,
        tc.tile_pool(name="pb_stage", bufs=2) as pb_stage,
    ):
        stages = [None] * RT
        ttTs = {}

        def emit_logit(c: int):
            g, pos = divmod(c, GRP)
            ttT = ttTs.pop(c)
            for rt in range(RT):
                pm = pb_psM.tile([128, CHW], F32, tag="pm")
                for kc in range(KD):
                    nc.tensor.matmul(
                        pm[:],
                        imgTb[:, kc, rt * 128:(rt + 1) * 128],
                        ttT[:, kc],
                        start=(kc == 0), stop=(kc == KD - 1),
                    )
                if pos == 0:
                    stages[rt] = pb_stage.tile(
                        [128, GRP * CHW], F32, tag=f"stg{rt}", name=f"stg{rt}g{g}")
                selb = sels[rt][:, c * (CHW // CPT):(c + 1) * (CHW // CPT)]
                selb = selb.broadcast_to([128, CHW // CPT, CPT])
                dst = stages[rt][:, pos * CHW:(pos + 1) * CHW]
                nc.vector.tensor_tensor(
                    dst.rearrange("p (a b) -> p a b", b=CPT),
                    pm[:].rearrange("p (a b) -> p a b", b=CPT),
                    selb, op=OP.mult)
                if pos == GRP - 1:
                    nc.sync.dma_start(
                        out[rt * 128:(rt + 1) * 128,
                            g * GRP * CHW:(g + 1) * GRP * CHW],
                        stages[rt][:])

        for c in range(NCH):
            if c + 2 < NCH:
                rcps[c + 2] = emit_norms(c + 2)
            if c % 2 == 0 and c // 2 + 4 < NUNIT:
                load_unit(c // 2 + 4)

            u, half = divmod(c, 2)
            traw = traw_tiles[u]
            toff = half * TPC
            rcp = rcps.pop(c)

            # Per-tile diag(rcp) on GPSIMD, then a REGULAR matmul computes the
            # scaled transpose: ttT[:, kc, tile] = traw_tile[:, kc].T @ diag.
            # (PE transpose-mode ignores the values of its moving operand, so
            # it cannot apply the scaling; a plain matmul at the same cost can.)
            # Padded to 512/kc-row (one full PSUM bank) so no matmul output
            # crosses a 2KB bank boundary (crossing corrupts the write).
            ttT_ps = pb_ttT_ps.tile([128, KD, 512], F32, tag="ttps")
            for t in range(TPC):
                diag = pb_diag.tile([CT, CT], BF16, tag="diag")
                nc.vector.tensor_scalar(
                    diag[:], identb[:], rcp[:, t:t + 1], None, op0=OP.mult)
                for kc in range(KD):
                    nc.tensor.matmul(
                        ttT_ps[:, kc, t * CT:(t + 1) * CT],
                        traw[:, toff + t, kc * 128:(kc + 1) * 128],
                        diag[:], start=True, stop=True)
            ttT = pb_ttT.tile([128, KD, CHW], BF16, tag="ttT")
            # f32 PSUM -> bf16 SBUF cast-copy; alternate ACT/DVE for balance.
            if c % 2 == 0:
                nc.scalar.copy(ttT[:], ttT_ps[:, :, :CHW])
            else:
                nc.vector.tensor_copy(ttT[:], ttT_ps[:, :, :CHW])
            ttTs[c] = ttT

            # Lag the logit matmuls one chunk behind the transposes so the
            # PE never waits on the DVE drain of the chunk it just produced.
            if c > 0:
                emit_logit(c - 1)
        emit_logit(NCH - 1)


def _build(k: int, inv_s2: float):
    nc = bacc.Bacc("TRN2", target_bir_lowering=False, debug=False)
    img = nc.dram_tensor("img", [RLOC, D], F32, kind="ExternalInput").ap()
    proto = nc.dram_tensor("proto", [NP, D], F32, kind="ExternalInput").ap()
    text = nc.dram_tensor("text", [NC, D], F32, kind="ExternalInput").ap()
    out = nc.dram_tensor("out", [RLOC, NC], F32, kind="ExternalOutput").ap()
    with tile.TileContext(nc) as tc:
        with ExitStack() as ctx:
            _emit(ctx, tc, img, proto, text, out, k, inv_s2)
    nc.compile()
    return nc


def kernel(image_features, ima_proto, text_features_raw, logit_scale, num_test):
    global LAST_RESULTS
    img = np.ascontiguousarray(np.asarray(image_features, dtype=np.float32))
    proto = np.ascontiguousarray(np.asarray(ima_proto, dtype=np.float32))
    text = np.ascontiguousarray(np.asarray(text_features_raw, dtype=np.float32))
    assert img.shape == (B, D) and proto.shape == (NP, D) and text.shape == (NC, D)
    s = float(np.asarray(logit_scale))
    k = min(int(np.asarray(num_test)), NP)
    assert 1 <= k <= 16, f"kernel supports k in [1, 16], got {k}"
    inv_s2 = float(np.exp(-2.0 * s))

    nc = _build(k, inv_s2)
    in_maps = [
        {"img": img[i * RLOC:(i + 1) * RLOC], "proto": proto, "text": text}
        for i in range(NCORES)
    ]
    trace = bool(int(os.environ.get("BASS_KERNEL_TRACE", "0")))
    res = run_bass_kernel_spmd(nc, in_maps, list(range(NCORES)), trace=trace)
    LAST_RESULTS = res
    return np.concatenate([r["out"] for r in res.results], axis=0)


# revision 19
# speedup vs baseline: 1.5915x; 1.1079x over previous
"""Trainium2 Bass kernel for nn_CLIP topk_masking.

Computes, for full inputs (self-contained; shapes hardcoded):
    probability = image_features @ ima_proto.T          # [B, NP]
    thr_r       = k-th largest of probability row r
    sel[r, j]   = probability[r, j] >= thr_r            # top-k prototype mask
    text_n      = exp(logit_scale) * text_raw / ||text_raw||_row
    logits[r,c] = (image_features @ text_n.T)[r,c] * sel[r, c // 10]

Sharding: data-parallel over the batch axis across 8 NeuronCores
(rows 512/core); prototypes and text features replicated.

Design notes:
  - Text SBUF tiles use 100 partitions (10000 = 100*100) so DMAs fan out
    across 10 SDMA engines (the engine split is a divisor of the
    partition count; 125-partition tiles only got 5 engines).
  - Text is cast f32->bf16 during the SWDGE load; the whole text pipeline
    (norms, scale, transpose, logit matmul) runs in bf16.
  - Per-class normalization (with exp(logit_scale)) is folded into the PE
    transpose, done as a REGULAR matmul against diag(rcp): PE
    transpose-mode ignores its moving operand's values, a plain matmul
    at identical cost applies the scaling for real.
  - Transpose outputs land in an f32 PSUM tile padded to 512/kc-row so no
    matmul output crosses a 2KB PSUM bank boundary (crossing corrupts).
  - Row norms on ACT (Square + accumulator; one activation table), diag
    build + mask-apply (PSUM drain) on DVE, ttT drains split DVE/ACT.
  - Output stores on the HWDGE (sync) queue, text loads on SWDGE behind
    the img load; proto load on HWDGE.
"""

import os
from contextlib import ExitStack

import numpy as np

import concourse.bass as bass
import concourse.tile as tile
from concourse import bacc, mybir
from concourse.bass_utils import run_bass_kernel_spmd

# Problem shapes (hardcoded per contract).
B, D, NP, NC, CPT = 4096, 512, 1000, 10000, 10
NCORES = 8
RLOC = B // NCORES          # 512 rows per core
RT = RLOC // 128            # 4 row tiles per core
KD = D // 128               # 4 contraction chunks
CT = 100                    # text classes per tile
TPC = 5                     # text tiles per chunk
CHW = CT * TPC              # 500 classes per logit-matmul chunk
NCH = NC // CHW             # 20 chunks
GRP = 4                     # chunks per output stage group (2000 cols/store)
CTP = 100                   # proto classes per tile
UNIT = 1000                 # text classes per load unit (2 chunks)
NUNIT = NC // UNIT          # 10 uniform units
NEG = -1.0e30

F32 = mybir.dt.float32
BF16 = mybir.dt.bfloat16

LAST_RESULTS = None


def _emit(ctx: ExitStack, tc, img, proto, text, out, k: int, inv_s2: float):
    nc = tc.nc
    AF = mybir.ActivationFunctionType
    OP = mybir.AluOpType

    const = ctx.enter_context(tc.tile_pool(name="const", bufs=1))
    persist = ctx.enter_context(tc.tile_pool(name="persist", bufs=1))

    # Identity matrices: f32 for img/proto PE transposes, bf16 replica
    # block for the per-chunk diag(rcp) build.
    ones = const.tile([128, 128], F32)
    nc.vector.memset(ones[:], 1.0)
    ident = const.tile([128, 128], F32)
    nc.gpsimd.affine_select(
        ident[:], ones[:], pattern=[[1, 128]], compare_op=OP.is_equal,
        fill=0.0, base=0, channel_multiplier=-1,
    )
    identb5 = const.tile([CT, TPC, CT], BF16)
    for t in range(TPC):
        nc.vector.tensor_copy(identb5[:, t], ident[:CT, :CT])

    # imgT[p, kc, r] = img[r, kc*128 + p]; f32 for the prob matmul, bf16
    # copy for the logit matmul. sel[rt] = top-k prototype mask per row.
    imgT = persist.tile([128, KD, RLOC], F32)
    imgTb = persist.tile([128, KD, RLOC], BF16)
    sels = []

    # Text loads: SWDGE (gpsimd) with f32->bf16 cast. 5 buffers so a load
    # never waits on its buffer's previous consumers (no Pool stalls).
    pb_traw = ctx.enter_context(tc.tile_pool(name="pb_traw", bufs=5))
    traw_tiles = {}

    def load_unit(u: int):
        t_ = pb_traw.tile([CT, UNIT // CT, D], BF16, name=f"traw{u}", tag="traw")
        nc.gpsimd.dma_start(
            t_[:], text[u * UNIT:(u + 1) * UNIT].rearrange(
                "(t p) d -> p t d", p=CT))
        traw_tiles[u] = t_

    # Long-lived text-pipeline pools (PSUM: ttT = 4 banks single-buffered).
    pb_nrm = ctx.enter_context(tc.tile_pool(name="pb_nrm", bufs=3))
    pb_diag = ctx.enter_context(tc.tile_pool(name="pb_diag", bufs=2))
    pb_ttT_ps = ctx.enter_context(
        tc.tile_pool(name="pb_ttT_ps", bufs=1, space="PSUM"))
    pb_ttT = ctx.enter_context(tc.tile_pool(name="pb_ttT", bufs=3))
    dump = const.tile([CT, D], BF16)   # ACT activation main-out scratch

    # ---------- Phase A: img/proto transpose, probability matmul, top-k ----------
    with (
        tc.tile_pool(name="pa_sb", bufs=1) as pa_sb,
        tc.tile_pool(name="pa_ps", bufs=2, space="PSUM") as pa_ps,
        tc.tile_pool(name="pa_prob_ps", bufs=2, space="PSUM") as pa_prob_ps,
        tc.tile_pool(name="pa_work", bufs=2) as pa_work,
    ):
        # img ahead of the text units on SWDGE (small, needed first);
        # proto alone on the HWDGE (sync) queue so both arrive early.
        img_sb = pa_sb.tile([128, RT, D], F32)
        nc.gpsimd.dma_start(img_sb[:], img.rearrange("(t p) d -> p t d", p=128))
        for rt in range(RT):
            pi = pa_ps.tile([128, KD, 128], F32, tag="pt")
            for kc in range(KD):
                nc.tensor.transpose(
                    pi[:, kc], img_sb[:, rt, kc * 128:(kc + 1) * 128], ident[:])
            nc.scalar.copy(imgT[:, :, rt * 128:(rt + 1) * 128], pi[:])
        nc.vector.tensor_copy(imgTb[:], imgT[:])

        proto_sb = pa_sb.tile([CTP, NP // CTP, D], F32)
        nc.sync.dma_start(proto_sb[:], proto.rearrange("(t p) d -> p t d", p=CTP))
        protoT = pa_sb.tile([128, KD, NP], F32)
        for t in range(NP // CTP):
            pp = pa_ps.tile([128, KD, 128], F32, tag="pt")
            for kc in range(KD):
                nc.tensor.transpose(
                    pp[:, kc, :CTP], proto_sb[:, t, kc * 128:(kc + 1) * 128],
                    ident[:CTP, :CTP])
            nc.scalar.copy(
                protoT[:, :, t * CTP:(t + 1) * CTP], pp[:, :, :CTP])

        for rt in range(RT):
            prob = pa_work.tile([128, NP], F32, tag="prob")
            for h in range(2):
                ppr = pa_prob_ps.tile([128, NP // 2], F32, tag="ppr")
                for kc in range(KD):
                    # fp32 (not bf16): ranking precision decides the mask.
                    nc.tensor.matmul(
                        ppr[:],
                        imgT[:, kc, rt * 128:(rt + 1) * 128],
                        protoT[:, kc, h * (NP // 2):(h + 1) * (NP // 2)],
                        start=(kc == 0), stop=(kc == KD - 1),
                    )
                nc.scalar.copy(prob[:, h * (NP // 2):(h + 1) * (NP // 2)], ppr[:])
            m8a = pa_work.tile([128, 8], F32, tag="m8a")
            nc.vector.max(m8a[:], prob[:])
            if k <= 8:
                thr = m8a[:, k - 1:k]
            else:
                repl = pa_work.tile([128, NP], F32, tag="repl")
                nc.vector.match_replace(repl[:], m8a[:], prob[:], NEG)
                m8b = pa_work.tile([128, 8], F32, tag="m8b")
                nc.vector.max(m8b[:], repl[:])
                thr = m8b[:, k - 9:k - 8]
            sel = persist.tile([128, NP], F32, tag=f"sel{rt}")
            nc.vector.tensor_scalar(sel[:], prob[:], thr, None, op0=OP.is_ge)
            sels.append(sel)

    # Text prefetch starts right behind the img load on the SWDGE queue.
    for u in range(4):
        load_unit(u)

    # ---------- Phase B: text norms, scaled transpose, logit matmul, mask ----------
    def emit_norms(c: int):
        u, half = divmod(c, 2)
        traw = traw_tiles[u]
        toff = half * TPC
        nrm = pb_nrm.tile([CT, TPC], F32, tag="nrm", name=f"nrm{c}")
        for t in range(TPC):
            nc.scalar.activation(
                dump[:], traw[:, toff + t], AF.Square,
                accum_out=nrm[:, t:t + 1])
        nrs = pb_nrm.tile([CT, TPC], F32, tag="nrs", name=f"nrs{c}")
        # sqrt(||t||^2 * exp(-2*logit_scale)) = ||t|| / exp(logit_scale)
        nc.scalar.activation(nrs[:], nrm[:], AF.Sqrt, scale=inv_s2)
        rcp = pb_nrm.tile([CT, TPC], F32, tag="rcp", name=f"rcp{c}")
        nc.vector.reciprocal(rcp[:], nrs[:])
        return rcp

    rcps = {0: emit_norms(0), 1: emit_norms(1)}

    with (
        tc.tile_pool(name="pb_psM", bufs=RT, space="PSUM") as pb_psM,
        tc.tile_pool(name="pb_stage", bufs=2) as pb_stage,
    ):
        stages = [None] * RT
        ttTs = {}

        def emit_logit(c: int):
            g, pos = divmod(c, GRP)
            ttT = ttTs.pop(c)
            for rt in range(RT):
                pm = pb_psM.tile([128, CHW], F32, tag="pm")
                for kc in range(KD):
                    nc.tensor.matmul(
                        pm[:],
                        imgTb[:, kc, rt * 128:(rt + 1) * 128],
                        ttT[:, kc],
                        start=(kc == 0), stop=(kc == KD - 1),
                    )
                if pos == 0:
                    stages[rt] = pb_stage.tile(
                        [128, GRP * CHW], F32, tag=f"stg{rt}", name=f"stg{rt}g{g}")
                selb = sels[rt][:, c * (CHW // CPT):(c + 1) * (CHW // CPT)]
                selb = selb.broadcast_to([128, CHW // CPT, CPT])
                dst = stages[rt][:, pos * CHW:(pos + 1) * CHW]
                nc.vector.tensor_tensor(
                    dst.rearrange("p (a b) -> p a b", b=CPT),
                    pm[:].rearrange("p (a b) -> p a b", b=CPT),
                    selb, op=OP.mult)
                if pos == GRP - 1:
                    nc.sync.dma_start(
                        out[rt * 128:(rt + 1) * 128,
                            g * GRP * CHW:(g + 1) * GRP * CHW],
                        stages[rt][:])

        for c in range(NCH):
            if c + 2 < NCH:
                rcps[c + 2] = emit_norms(c + 2)
            if c % 2 == 0 and c // 2 + 4 < NUNIT:
                load_unit(c // 2 + 4)

            u, half = divmod(c, 2)
            traw = traw_tiles[u]
            toff = half * TPC
            rcp = rcps.pop(c)

            # One diag(rcp) block per chunk on DVE, then REGULAR matmuls
            # compute the scaled transpose: ttT = traw_tile.T @ diag(rcp).
            diag5 = pb_diag.tile([CT, TPC, CT], BF16, tag="diag")
            nc.vector.tensor_tensor(
                diag5[:], identb5[:], rcp[:].broadcast_to([CT, TPC, CT]),
                op=OP.mult)
            # Padded to 512/kc-row (one full PSUM bank) so no matmul output
            # crosses a 2KB bank boundary (crossing corrupts the write).
            ttT_ps = pb_ttT_ps.tile([128, KD, 512], F32, tag="ttps")
            for t in range(TPC):
                for kc in range(KD):
                    nc.tensor.matmul(
                        ttT_ps[:, kc, t * CT:(t + 1) * CT],
                        traw[:, toff + t, kc * 128:(kc + 1) * 128],
                        diag5[:, t], start=True, stop=True)
            ttT = pb_ttT.tile([128, KD, CHW], BF16, tag="ttT")
            # f32 PSUM -> bf16 SBUF drain; mostly DVE, every 3rd on ACT.
            if c % 3 == 2:
                nc.scalar.copy(ttT[:], ttT_ps[:, :, :CHW])
            else:
                nc.vector.tensor_copy(ttT[:], ttT_ps[:, :, :CHW])
            ttTs[c] = ttT

            # Lag the logit matmuls one chunk behind the transposes so the
            # PE never waits on the DVE drain of the chunk it just produced.
            if c > 0:
                emit_logit(c - 1)
        emit_logit(NCH - 1)


def _build(k: int, inv_s2: float):
    nc = bacc.Bacc("TRN2", target_bir_lowering=False, debug=False)
    img = nc.dram_tensor("img", [RLOC, D], F32, kind="ExternalInput").ap()
    proto = nc.dram_tensor("proto", [NP, D], F32, kind="ExternalInput").ap()
    text = nc.dram_tensor("text", [NC, D], F32, kind="ExternalInput").ap()
    out = nc.dram_tensor("out", [RLOC, NC], F32, kind="ExternalOutput").ap()
    with tile.TileContext(nc) as tc:
        with ExitStack() as ctx:
            _emit(ctx, tc, img, proto, text, out, k, inv_s2)
    nc.compile()
    return nc


def kernel(image_features, ima_proto, text_features_raw, logit_scale, num_test):
    global LAST_RESULTS
    img = np.ascontiguousarray(np.asarray(image_features, dtype=np.float32))
    proto = np.ascontiguousarray(np.asarray(ima_proto, dtype=np.float32))
    text = np.ascontiguousarray(np.asarray(text_features_raw, dtype=np.float32))
    assert img.shape == (B, D) and proto.shape == (NP, D) and text.shape == (NC, D)
    s = float(np.asarray(logit_scale))
    k = min(int(np.asarray(num_test)), NP)
    assert 1 <= k <= 16, f"kernel supports k in [1, 16], got {k}"
    inv_s2 = float(np.exp(-2.0 * s))

    nc = _build(k, inv_s2)
    in_maps = [
        {"img": img[i * RLOC:(i + 1) * RLOC], "proto": proto, "text": text}
        for i in range(NCORES)
    ]
    trace = bool(int(os.environ.get("BASS_KERNEL_TRACE", "0")))
    res = run_bass_kernel_spmd(nc, in_maps, list(range(NCORES)), trace=trace)
    LAST_RESULTS = res
    return np.concatenate([r["out"] for r in res.results], axis=0)
